# revision 13
# baseline (speedup 1.0000x reference)
"""Trainium2 Bass kernel for nn_NaiveBayes_80023830659567.

Reference math (B=500000, F=20 features, C=8 classes, std=0.3):
  feats = x[:, :20], mask = x[:, 20:]
  class c is Gaussian on features c, c+1, c+2 with means = bits of c
  (b0=c&1, b1=(c>>1)&1, b2=(c>>2)&1); all other features use a uniform
  [0,1) likelihood which is exactly 1 for these inputs, so only feat
  columns 0..9 and mask columns 20..29 matter.

  In log space, with K = INV_SQRT_2PI/std and q_m = ((x-m)/std)^2 / 2:
    L[b,c]  = sum_j mask[b,c+j] * (ln K - q_{bj}[b,c+j])
    probs   = exp(L);   out = probs / max(sum_c probs, 8e-8)
  (the 1/C factor cancels in the normalization; the 1e-8 clamp on
  0.125*sum becomes 8e-8 on sum).

Sharding: pure data parallel over the batch axis across 8 cores.
Each core runs an identical SPMD program on a [62720, 40] shard
(batch padded 500000 -> 501760 = 8*128*490 so every partition gets
the same whole number of rows).
"""

import math

import numpy as np

BATCH = 500000
N_CORES = 8
P = 128
RPP = 490            # rows per partition per core
TILES = (98, 98, 98, 98, 98)   # rows per partition per tile; sum == RPP
ROWS_PER_CORE = P * RPP          # 62720
PADDED = N_CORES * ROWS_PER_CORE  # 501760

STD = 0.3
INV_SQRT_2PI = 0.3989422804014327
A = 1.0 / (STD * math.sqrt(2.0))        # q = (x*A + m*A)^2 = ((x-m)/std)^2/2
LNK = math.log(INV_SQRT_2PI / STD)

_RUNNER = None


def _legalize_waits(nc, mybir):
    """This walrus build accepts at most 1 sync wait per instruction (2 for
    EventSemaphore). The Tile end-of-kernel drain can carry more; hoist the
    extras onto single-wait NoOps inserted just before the offender."""
    n_split = 0
    for f in nc.m.functions:
        for bb in f.blocks:
            lst = bb.instructions
            out = []
            changed = False
            for ins in lst:
                si = ins.sync_info
                cap = 2 if isinstance(ins, mybir.InstEventSemaphore) else 1
                if si is not None and si.on_wait and len(si.on_wait) > cap:
                    waits = list(si.on_wait)
                    for w in waits[:-cap]:
                        nop = mybir.InstNoOp(
                            name=nc.get_next_instruction_name(), ins=[], outs=[]
                        )
                        nop.engine = ins.engine
                        nop.sync_info = mybir.SyncInfo(on_wait=[w], on_update=[])
                        out.append(nop)
                        n_split += 1
                    ins.sync_info = mybir.SyncInfo(
                        on_wait=list(waits[-cap:]), on_update=list(si.on_update)
                    )
                    changed = True
                out.append(ins)
            if changed:
                lst.clear()
                lst.extend(out)
    return n_split


def _free_ap(base, extra_offset, free_dims):
    """AP over `base`'s tensor with base's partition dim and custom free dims."""
    import concourse.bass as bass

    return bass.AP(
        tensor=base.tensor,
        offset=base.offset + extra_offset,
        ap=[list(base.ap[0])] + [list(d) for d in free_dims],
    )


def _build_program(rep=1, tiles=TILES, mul_on_pool=True, stt_on_pool=True,
                   clamp=False):
    import concourse.bass as bass
    import concourse.tile as tile
    from concourse import mybir

    assert sum(tiles) == RPP
    f32 = mybir.dt.float32
    nc = bass.Bass()
    x_ext = nc.declare_dram_parameter("x", [ROWS_PER_CORE, 40], f32, isOutput=False)
    o_ext = nc.declare_dram_parameter("out", [ROWS_PER_CORE, 8], f32, isOutput=True)

    xv = x_ext[:].rearrange("(p r) f -> p r f", p=P)   # [128, 490, 40]
    ov = o_ext[:].rearrange("(p r) c -> p r c", p=P)   # [128, 490, 8]

    with tile.TileContext(nc) as tc:
        with (
            tc.tile_pool(name="xp", bufs=3) as xp,
            tc.tile_pool(name="qp", bufs=2) as qp,
            tc.tile_pool(name="vp", bufs=2) as vp,
            tc.tile_pool(name="lp", bufs=2) as lp,
            tc.tile_pool(name="pp", bufs=2) as pp,
            tc.tile_pool(name="sp", bufs=2) as sp,
            tc.tile_pool(name="ob", bufs=3) as ob,
            tc.tile_pool(name="consts", bufs=1) as consts,
        ):
            bias_mA = consts.tile([P, 1], f32)
            nc.vector.memset(bias_mA[:], -A)

            starts = [sum(tiles[:i]) for i in range(len(tiles))]
            for it in range(len(tiles) * rep):
                it = it % len(tiles)
                R = tiles[it]
                rows = slice(starts[it], starts[it] + R)
                X = xp.tile([P, R, 40], f32, tag="x")
                nc.sync.dma_start(out=X[:], in_=xv[:, rows, :])

                F = X[:, :, 0:10]
                M = X[:, :, 20:30]

                # q_m = ((x - m)/std)^2 / 2 for m in {0, 1}
                q0 = qp.tile([P, R, 10], f32, tag="q0")
                q1 = qp.tile([P, R, 10], f32, tag="q1")
                Sq = mybir.ActivationFunctionType.Square
                nc.scalar.activation(q0[:], F, Sq, bias=0.0, scale=A)
                nc.scalar.activation(q1[:], F, Sq, bias=bias_mA[:], scale=A)

                # V[., r, f, m] = (q_m[r,f] - lnK) * mask[r,f]   (= -log factor)
                V = vp.tile([P, R, 10, 2], f32)
                w1_eng = nc.gpsimd if stt_on_pool else nc.vector
                nc.vector.scalar_tensor_tensor(
                    out=V[:, :, :, 0], in0=q0[:], scalar=LNK, in1=M,
                    op0=mybir.AluOpType.subtract, op1=mybir.AluOpType.mult,
                )
                w1_eng.scalar_tensor_tensor(
                    out=V[:, :, :, 1], in0=q1[:], scalar=LNK, in1=M,
                    op0=mybir.AluOpType.subtract, op1=mybir.AluOpType.mult,
                )

                # L'[., r, c] = sum_j V[., r, c+j, b_j(c)]  (= -L)
                # inner-offset patterns within each 20-element (f,m) block,
                # with c = 4a+2b+d:
                #   j=0: 2c+b0   = 4(2a+b)+3d
                #   j=1: 2c+2+b1 = 2+8a+5b+2d
                #   j=2: 2c+4+b2 = 4+9a+4b+2d
                # DVE APs allow at most 3 free dims, so do each add in two
                # halves over the a bit (classes 0-3 / 4-7).
                vb = V[:]
                L = lp.tile([P, R, 8], f32)
                for a in range(2):
                    G0 = _free_ap(vb, 8 * a, [[20, R], [4, 2], [3, 2]])
                    G1 = _free_ap(vb, 2 + 8 * a, [[20, R], [5, 2], [2, 2]])
                    G2 = _free_ap(vb, 4 + 9 * a, [[20, R], [4, 2], [2, 2]])
                    Lh = L[:, :, 4 * a : 4 * a + 4]
                    nc.vector.tensor_add(Lh, G0, G1)
                    nc.vector.tensor_add(Lh, Lh, G2)

                # probs = exp(L) = exp(-L')
                Pr = pp.tile([P, R, 8], f32)
                nc.scalar.activation(
                    Pr[:], L[:], mybir.ActivationFunctionType.Exp,
                    bias=0.0, scale=-1.0,
                )

                # normalizer (S >= ~1.1e-6 always, so the reference's 1e-8
                # clamp can never fire; keep it optional)
                S = sp.tile([P, R], f32, tag="s")
                nc.vector.tensor_reduce(
                    out=S[:], in_=Pr[:], axis=mybir.AxisListType.X,
                    op=mybir.AluOpType.add,
                )
                if clamp:
                    nc.vector.tensor_scalar_max(S[:], S[:], 8e-8)
                Rcp = sp.tile([P, R], f32, tag="rcp")
                nc.vector.reciprocal(out=Rcp[:], in_=S[:])

                # out = probs * (1/S)  (broadcast over the 8 classes)
                OB = ob.tile([P, R, 8], f32)
                rb = Rcp[:]
                Rb = _free_ap(rb, 0, [[1, R], [0, 8]])
                mul_eng = nc.gpsimd if mul_on_pool else nc.vector
                mul_eng.tensor_mul(OB[:], Pr[:], Rb)

                nc.sync.dma_start(out=ov[:, rows, :], in_=OB[:])

    _legalize_waits(nc, mybir)
    return nc


def _get_runner():
    global _RUNNER
    if _RUNNER is None:
        from concourse.bass_utils import run_bass_kernel_spmd

        nc = _build_program()

        def run(shards, **kw):
            in_maps = [{"x": s} for s in shards]
            return run_bass_kernel_spmd(nc, in_maps, list(range(N_CORES)), **kw)

        _RUNNER = run
    return _RUNNER


def _shard(x):
    x = np.ascontiguousarray(np.asarray(x, dtype=np.float32))
    assert x.shape == (BATCH, 40), x.shape
    xp = np.zeros((PADDED, 40), dtype=np.float32)
    xp[:BATCH] = x
    return [
        xp[i * ROWS_PER_CORE : (i + 1) * ROWS_PER_CORE] for i in range(N_CORES)
    ]


def kernel_raw(x, **kw):
    """Run the SPMD kernel; returns (full_output, BassKernelResults)."""
    res = _get_runner()(_shard(x), **kw)
    out = np.concatenate([res.results[i]["out"] for i in range(N_CORES)], axis=0)
    return out[:BATCH], res


def kernel(x):
    return kernel_raw(x)[0]


# revision 16
# speedup vs baseline: 2.5525x; 2.5525x over previous
"""Trainium2 Bass kernel for nn_NaiveBayes_80023830659567.

Reference math (B=500000, F=20 features, C=8 classes, std=0.3):
  feats = x[:, :20], mask = x[:, 20:]
  class c is Gaussian on features c, c+1, c+2 with means = bits of c
  (b0=c&1, b1=(c>>1)&1, b2=(c>>2)&1); all other features use a uniform
  [0,1) likelihood which is exactly 1 for these inputs, so only feat
  columns 0..9 and mask columns 20..29 matter.

  In log space, with K = INV_SQRT_2PI/std and q_m = ((x-m)/std)^2 / 2:
    L[b,c]  = sum_j mask[b,c+j] * (ln K - q_{bj}[b,c+j])
    probs   = exp(L);   out = probs / max(sum_c probs, 8e-8)
  (the 1/C factor cancels in the normalization; the 1e-8 clamp on
  0.125*sum becomes 8e-8 on sum).

Sharding: pure data parallel over the batch axis across 8 cores.
Each core runs an identical SPMD program on a [62720, 40] shard
(batch padded 500000 -> 501760 = 8*128*490 so every partition gets
the same whole number of rows).
"""

import math

import numpy as np

BATCH = 500000
N_CORES = 8
P = 128
RPP = 490            # rows per partition per core
# rows per partition per tile; sum == RPP. Tapered tail: the last tiles
# are small so the post-DMA compute chain and final out-DMA are short.
TILES = (56, 56, 56, 56, 56, 56, 56, 42, 35, 21)
ROWS_PER_CORE = P * RPP          # 62720
PADDED = N_CORES * ROWS_PER_CORE  # 501760

STD = 0.3
INV_SQRT_2PI = 0.3989422804014327
A = 1.0 / (STD * math.sqrt(2.0))        # q = (x*A + m*A)^2 = ((x-m)/std)^2/2
LNK = math.log(INV_SQRT_2PI / STD)

_RUNNER = None


def _legalize_waits(nc, mybir):
    """This walrus build accepts at most 1 sync wait per instruction (2 for
    EventSemaphore). The Tile end-of-kernel drain can carry more; hoist the
    extras onto single-wait NoOps inserted just before the offender."""
    n_split = 0
    for f in nc.m.functions:
        for bb in f.blocks:
            lst = bb.instructions
            out = []
            changed = False
            for ins in lst:
                si = ins.sync_info
                cap = 2 if isinstance(ins, mybir.InstEventSemaphore) else 1
                if si is not None and si.on_wait and len(si.on_wait) > cap:
                    waits = list(si.on_wait)
                    for w in waits[:-cap]:
                        nop = mybir.InstNoOp(
                            name=nc.get_next_instruction_name(), ins=[], outs=[]
                        )
                        nop.engine = ins.engine
                        nop.sync_info = mybir.SyncInfo(on_wait=[w], on_update=[])
                        out.append(nop)
                        n_split += 1
                    ins.sync_info = mybir.SyncInfo(
                        on_wait=list(waits[-cap:]), on_update=list(si.on_update)
                    )
                    changed = True
                out.append(ins)
            if changed:
                lst.clear()
                lst.extend(out)
    return n_split


def _free_ap(base, extra_offset, free_dims):
    """AP over `base`'s tensor with base's partition dim and custom free dims."""
    import concourse.bass as bass

    return bass.AP(
        tensor=base.tensor,
        offset=base.offset + extra_offset,
        ap=[list(base.ap[0])] + [list(d) for d in free_dims],
    )


def _build_program(rep=1, tiles=TILES, mul_on_pool=False, stt_on_pool=False,
                   clamp=False):
    import concourse.bass as bass
    import concourse.tile as tile
    from concourse import mybir

    assert sum(tiles) == RPP
    f32 = mybir.dt.float32
    nc = bass.Bass()
    x_ext = nc.declare_dram_parameter("x", [ROWS_PER_CORE, 40], f32, isOutput=False)
    o_ext = nc.declare_dram_parameter("out", [ROWS_PER_CORE, 8], f32, isOutput=True)

    xv = x_ext[:].rearrange("(p r) f -> p r f", p=P)   # [128, 490, 40]
    ov = o_ext[:].rearrange("(p r) c -> p r c", p=P)   # [128, 490, 8]

    with tile.TileContext(nc) as tc:
        with (
            tc.tile_pool(name="xp", bufs=3) as xp,
            tc.tile_pool(name="qp", bufs=2) as qp,
            tc.tile_pool(name="vp", bufs=2) as vp,
            tc.tile_pool(name="lp", bufs=2) as lp,
            tc.tile_pool(name="pp", bufs=2) as pp,
            tc.tile_pool(name="sp", bufs=2) as sp,
            tc.tile_pool(name="ob", bufs=3) as ob,
            tc.tile_pool(name="consts", bufs=1) as consts,
        ):
            bias_mA = consts.tile([P, 1], f32)
            nc.vector.memset(bias_mA[:], -A)

            starts = [sum(tiles[:i]) for i in range(len(tiles))]
            for it in range(len(tiles) * rep):
                it = it % len(tiles)
                R = tiles[it]
                rows = slice(starts[it], starts[it] + R)
                X = xp.tile([P, R, 40], f32, tag="x")
                nc.sync.dma_start(out=X[:], in_=xv[:, rows, :])

                F = X[:, :, 0:10]
                M = X[:, :, 20:30]

                # q_m = ((x - m)/std)^2 / 2 for m in {0, 1}
                q0 = qp.tile([P, R, 10], f32, tag="q0")
                q1 = qp.tile([P, R, 10], f32, tag="q1")
                Sq = mybir.ActivationFunctionType.Square
                nc.scalar.activation(q0[:], F, Sq, bias=0.0, scale=A)
                nc.scalar.activation(q1[:], F, Sq, bias=bias_mA[:], scale=A)

                # V[., r, f, m] = (q_m[r,f] - lnK) * mask[r,f]   (= -log factor)
                V = vp.tile([P, R, 10, 2], f32)
                w1_eng = nc.gpsimd if stt_on_pool else nc.vector
                nc.vector.scalar_tensor_tensor(
                    out=V[:, :, :, 0], in0=q0[:], scalar=LNK, in1=M,
                    op0=mybir.AluOpType.subtract, op1=mybir.AluOpType.mult,
                )
                w1_eng.scalar_tensor_tensor(
                    out=V[:, :, :, 1], in0=q1[:], scalar=LNK, in1=M,
                    op0=mybir.AluOpType.subtract, op1=mybir.AluOpType.mult,
                )

                # L'[., r, c] = sum_j V[., r, c+j, b_j(c)]  (= -L)
                # inner-offset patterns within each 20-element (f,m) block,
                # with c = 4a+2b+d:
                #   j=0: 2c+b0   = 4(2a+b)+3d
                #   j=1: 2c+2+b1 = 2+8a+5b+2d
                #   j=2: 2c+4+b2 = 4+9a+4b+2d
                # DVE APs allow at most 3 free dims, so do each add in two
                # halves over the a bit (classes 0-3 / 4-7).
                vb = V[:]
                L = lp.tile([P, R, 8], f32)
                for a in range(2):
                    G0 = _free_ap(vb, 8 * a, [[20, R], [4, 2], [3, 2]])
                    G1 = _free_ap(vb, 2 + 8 * a, [[20, R], [5, 2], [2, 2]])
                    G2 = _free_ap(vb, 4 + 9 * a, [[20, R], [4, 2], [2, 2]])
                    Lh = L[:, :, 4 * a : 4 * a + 4]
                    nc.vector.tensor_add(Lh, G0, G1)
                    nc.vector.tensor_add(Lh, Lh, G2)

                # probs = exp(L) = exp(-L')
                Pr = pp.tile([P, R, 8], f32)
                nc.scalar.activation(
                    Pr[:], L[:], mybir.ActivationFunctionType.Exp,
                    bias=0.0, scale=-1.0,
                )

                # normalizer (S >= ~1.1e-6 always, so the reference's 1e-8
                # clamp can never fire; keep it optional)
                S = sp.tile([P, R], f32, tag="s")
                nc.vector.tensor_reduce(
                    out=S[:], in_=Pr[:], axis=mybir.AxisListType.X,
                    op=mybir.AluOpType.add,
                )
                if clamp:
                    nc.vector.tensor_scalar_max(S[:], S[:], 8e-8)
                Rcp = sp.tile([P, R], f32, tag="rcp")
                nc.vector.reciprocal(out=Rcp[:], in_=S[:])

                # out = probs * (1/S)  (broadcast over the 8 classes)
                OB = ob.tile([P, R, 8], f32)
                rb = Rcp[:]
                Rb = _free_ap(rb, 0, [[1, R], [0, 8]])
                mul_eng = nc.gpsimd if mul_on_pool else nc.vector
                mul_eng.tensor_mul(OB[:], Pr[:], Rb)

                nc.sync.dma_start(out=ov[:, rows, :], in_=OB[:])

    _legalize_waits(nc, mybir)
    return nc


def _get_runner():
    global _RUNNER
    if _RUNNER is None:
        from concourse.bass_utils import run_bass_kernel_spmd

        nc = _build_program()

        def run(shards, **kw):
            in_maps = [{"x": s} for s in shards]
            return run_bass_kernel_spmd(nc, in_maps, list(range(N_CORES)), **kw)

        _RUNNER = run
    return _RUNNER


def _shard(x):
    x = np.ascontiguousarray(np.asarray(x, dtype=np.float32))
    assert x.shape == (BATCH, 40), x.shape
    xp = np.zeros((PADDED, 40), dtype=np.float32)
    xp[:BATCH] = x
    return [
        xp[i * ROWS_PER_CORE : (i + 1) * ROWS_PER_CORE] for i in range(N_CORES)
    ]


def kernel_raw(x, **kw):
    """Run the SPMD kernel; returns (full_output, BassKernelResults)."""
    res = _get_runner()(_shard(x), **kw)
    out = np.concatenate([res.results[i]["out"] for i in range(N_CORES)], axis=0)
    return out[:BATCH], res


def kernel(x):
    return kernel_raw(x)[0]
